# revision 12
# baseline (speedup 1.0000x reference)
"""Trainium2 Bass kernel for nn_CrossAttention (B=2, N=2048, D=1024, H=16).

v3: K-projection replicated per core (interleaved with attention waves
so PE never starves); V-projection token-sharded and AllGathered over
all 8 cores (mesh path, ~28us for 1MB/rank vs ~55us for the 4-rank
ring), with a per-core batch-select mask applied at readback (the
8-core gather returns both batches' shards; out = b0 + (b1-b0)*mask).

Core c -> (batch b = c//4, slice g = c%4): owns 512 query tokens and
512 V key tokens.  Attention computes all 16 heads for the own query
slice; output projection is local (no output collective).

PV matmuls in x-layout (out=[128 q, 65], lhsT=P^T tile, rhs=[V|ones]):
full PE rows, 65 f_r rows/instr vs 512 in the x^T layout.  Softmax
denominators ride along as the ones column; normalization is one DVE
reciprocal + per-partition scalar multiplies; x^T for the output
projection comes from PE transposes (two heads packed into one PSUM
bank via output base-partition 0/64).

PSUM has_written is cleared BANK-wide by matmul start=True, so only
the first matmul touching an accumulator bank per wave carries start.

Exp on ACT engine [128, 1024]/instr (~1.07us x 128) paces the
attention phase; a deep pt buffer (8 tiles) lets exp run ahead while
the V gather lands.
"""

import numpy as np

B = 2
NT = 2048
D = 1024
HEADS = 16
DH = 64
NCORES = 8
CPB = 4            # cores per batch group
QSL = NT // CPB    # 512 own query tokens per core
KSL = NT // CPB    # 512 own V key tokens per core
VW = DH + 1        # V columns per head incl. ones column
GROUPS_ALL = [[0, 1, 2, 3, 4, 5, 6, 7]]
SCALE = DH ** -0.5
NKT = NT // 128    # 16 key-token tiles

_patched = False


def _patch_tile_drain():
    """This container's walrus rejects >1 sync-wait on a Drain
    (CoreV3GenImpl setupSyncWait<CTRL_NO_STRUCT>: "Too many sync wait
    commands").  Split the final TileContext drain's waits across a chain
    of single-wait drains; semaphores are monotonic so sequential waits
    are equivalent to one multi-wait."""
    global _patched
    if _patched:
        return
    import concourse.tile as tile
    import concourse.mybir as mybir
    from concourse.vector_clock import ScopedClock

    _uid = [0]

    def _split_multiwaits(nc):
        for f in nc.m.functions:
            for bb in f.blocks:
                il = bb.instructions
                i = 0
                while i < len(il):
                    inst = il[i]
                    si = inst.sync_info
                    if si is not None and len(si.on_wait) > 1:
                        waits = list(si.on_wait)
                        inst.sync_info = mybir.SyncInfo(
                            on_wait=[waits[-1]], on_update=list(si.on_update)
                        )
                        for w in waits[:-1]:
                            _uid[0] += 1
                            nop = mybir.InstEventSemaphore(
                                name=f"WSPLIT-{_uid[0]}",
                                engine=inst.engine,
                                ins=[],
                                outs=[],
                                sync_info=mybir.SyncInfo(
                                    on_wait=[w], on_update=[]),
                            )
                            il.insert(i, nop)
                            i += 1
                    i += 1

    def _drain_and_barrier(self, tick_clock, wait_clock):
        nc = self.nc
        drain_inst = nc.sync.drain()
        wait_clock.add_sem_waits(
            drain_inst.ins, ScopedClock({None: tick_clock.global_clock})
        )
        si = drain_inst.ins.sync_info
        if si is not None and len(si.on_wait) > 1:
            waits = list(si.on_wait)
            drain_inst.ins.sync_info = mybir.SyncInfo(
                on_wait=[waits[0]], on_update=list(si.on_update)
            )
            for w in waits[1:]:
                extra = nc.sync.drain()
                extra.ins.sync_info = mybir.SyncInfo(on_wait=[w], on_update=[])

        _split_multiwaits(nc)
        nc.all_engine_barrier()
        assert self.sems is not None
        popped = nc._tile_sem_poison_stack.pop()
        assert popped is self._sem_poison
        nc.clear_and_free_semaphores(list(self.sems.allocated().values()))
        nc.all_engine_barrier()

    tile.TileContext._drain_and_barrier = _drain_and_barrier
    _patched = True


def build_program():
    """Build the SPMD Bass program (rank-uniform, one NeuronCore's view)."""
    _patch_tile_drain()
    import concourse.bass as bass
    import concourse.tile as tile
    import concourse.mybir as mybir
    from concourse.masks import make_identity

    f32 = mybir.dt.float32
    bf16 = mybir.dt.bfloat16
    EXP = mybir.ActivationFunctionType.Exp
    MUL = mybir.AluOpType.mult
    ADD = mybir.AluOpType.add
    SUB = mybir.AluOpType.subtract

    nc = bass.Bass("TRN2", target_bir_lowering=False, debug=False,
                   num_devices=NCORES)

    # pre-shaped host-side: [128, 8, X] with row (t p) -> [p, t, :]
    xq3 = nc.dram_tensor("xq3", [128, 8, QSL], bf16, kind="ExternalInput")
    xk3 = nc.dram_tensor("xk3", [128, 8, NT], bf16, kind="ExternalInput")
    xv3 = nc.dram_tensor("xv3", [128, 8, KSL], bf16, kind="ExternalInput")
    wq3 = nc.dram_tensor("wq3", [128, 8, D], bf16, kind="ExternalInput")
    wk3 = nc.dram_tensor("wk3", [128, 8, D], bf16, kind="ExternalInput")
    wv3 = nc.dram_tensor("wv3", [128, 8, D], bf16, kind="ExternalInput")
    wo3 = nc.dram_tensor("wo3", [128, 8, D], bf16, kind="ExternalInput")
    bo = nc.dram_tensor("bo", [D], f32, kind="ExternalInput")
    bsel = nc.dram_tensor("bsel", [1], f32, kind="ExternalInput")
    out = nc.dram_tensor("out", [QSL, D], f32, kind="ExternalOutput")

    with tile.TileContext(nc) as tc:
        from contextlib import ExitStack
        with ExitStack() as ctx:
            const = ctx.enter_context(tc.tile_pool(name="const", bufs=1))
            # weights rotate: wv -> A, wk -> B, wq -> C, wo -> A again
            wpool = ctx.enter_context(tc.tile_pool(name="wpool", bufs=3))
            xpool = ctx.enter_context(tc.tile_pool(name="xpool", bufs=1))
            persist = ctx.enter_context(tc.tile_pool(name="persist", bufs=1))
            kt_pool = ctx.enter_context(tc.tile_pool(name="ktp", bufs=2))
            vs_pool = ctx.enter_context(tc.tile_pool(name="vs", bufs=2))
            pt_pool = ctx.enter_context(tc.tile_pool(name="pt", bufs=8))
            xn_pool = ctx.enter_context(tc.tile_pool(name="xn", bufs=2))
            misc = ctx.enter_context(tc.tile_pool(name="misc", bufs=4))
            outsb = ctx.enter_context(tc.tile_pool(name="outsb", bufs=2))
            dram = ctx.enter_context(
                tc.tile_pool(name="dram", bufs=1, space="DRAM"))
            # PSUM: big_ps 2x [128,1024] (proj accs, score pairs) = 4 banks;
            # ps1 4x [128,512] (PV accumulators as [128,4,65] views,
            # transpose targets, out-proj finals) = 4 banks.
            big_ps = ctx.enter_context(
                tc.tile_pool(name="big_ps", bufs=2, space="PSUM"))
            ps1 = ctx.enter_context(
                tc.tile_pool(name="ps1", bufs=4, space="PSUM"))

            # --- warm the Exp activation table early ----------------------
            junk = const.tile([1, 8], f32)
            nc.vector.memset(junk[:], 0.0)
            jout = const.tile([1, 8], bf16)
            nc.scalar.activation(jout[:], junk[:], EXP)

            ident = const.tile([128, 128], bf16)
            make_identity(nc, ident)

            # --- input DMA: V-proj deps first (its AllGather is the gate),
            # then K/Q; wo/bias last.
            wv_sb = wpool.tile([128, 8, D], bf16, tag="w", name="wv_sb")
            xv_sb = xpool.tile([128, 8, KSL], bf16, tag="xv", name="xv_sb")
            nc.sync.dma_start(out=wv_sb[:], in_=wv3[:])
            nc.sync.dma_start(out=xv_sb[:], in_=xv3[:])
            msk = const.tile([128, 1], f32)
            nc.sync.dma_start(out=msk[:], in_=bsel[:].partition_broadcast(128))
            wk_sb = wpool.tile([128, 8, D], bf16, tag="w", name="wk_sb")
            xk_sb = xpool.tile([128, 8, NT], bf16, tag="xk", name="xk_sb")
            nc.sync.dma_start(out=wk_sb[:], in_=wk3[:])
            for k in range(8):
                nc.sync.dma_start(out=xk_sb[:, k, :], in_=xk3[:, k, :])
            wq_sb = wpool.tile([128, 8, D], bf16, tag="w", name="wq_sb")
            xq_sb = xpool.tile([128, 8, QSL], bf16, tag="xq", name="xq_sb")
            nc.sync.dma_start(out=wq_sb[:], in_=wq3[:])
            nc.sync.dma_start(out=xq_sb[:], in_=xq3[:])

            # --- persistent tiles -----------------------------------------
            v_sb = persist.tile([128, NKT, HEADS, VW], bf16)
            qt_sb = persist.tile([128, 8, QSL], bf16)
            xt_sb = persist.tile([128, 8, QSL], bf16)  # x^T normalized
            psb = persist.tile([128, 8, QSL], bf16)    # out-proj partials

            vag_s = vs_pool.tile([128, 4, HEADS, VW], bf16, tag="vs",
                                 name="vag_s")
            nc.vector.memset(vag_s[:, :, :, DH:], 1.0)

            vag_in = dram.tile([128, 4, HEADS, VW], bf16)
            vag_out = dram.tile([128 * NCORES, 4, HEADS, VW], bf16,
                                addr_space="Shared")

            # --- V projection (own 512 key tokens, all 16 heads) ----------
            for i in range(4):
                acc = big_ps.tile([128, 1024], f32, tag="mm", name="vacc")
                for n in range(2):
                    for k in range(8):
                        nc.tensor.matmul(
                            acc[:, 512 * n:512 * (n + 1)],
                            xv_sb[:, k, 128 * i:128 * (i + 1)],
                            wv_sb[:, k, 512 * n:512 * (n + 1)],
                            start=(k == 0), stop=(k == 7))
                nc.vector.tensor_copy(
                    vag_s[:, i, :, 0:DH],
                    acc[:].rearrange("p (h c) -> p h c", c=DH))
                nc.sync.dma_start(out=vag_in[:, i], in_=vag_s[:, i])

            # 8-core AllGather (mesh): returns BOTH batches' shards.
            nc.gpsimd.collective_compute(
                "AllGather", mybir.AluOpType.bypass,
                replica_groups=GROUPS_ALL,
                ins=[vag_in.opt()], outs=[vag_out.opt()])

            # Readback + batch select: v = b0 + (b1 - b0) * msk.
            # Emitted on the sync queue AFTER all input DMAs above so the
            # AG-completion wait does not block them.
            for r in range(CPB):
                vsl = v_sb[:, 4 * r:4 * (r + 1)]
                nc.sync.dma_start(out=vsl,
                                  in_=vag_out[128 * r:128 * (r + 1)])
                vtmp = vs_pool.tile([128, 4, HEADS, VW], bf16, tag="vs",
                                    name="vtmp")
                nc.sync.dma_start(
                    out=vtmp[:],
                    in_=vag_out[128 * (4 + r):128 * (5 + r)])
                nc.vector.tensor_tensor(vtmp[:], vtmp[:], vsl, SUB)
                nc.vector.scalar_tensor_tensor(
                    vsl, vtmp[:], msk[:], vsl, MUL, ADD)

            # --- attention waves (head pair w), K/Q chunks interleaved ----
            pending = None  # (w, xn) awaiting transpose emission

            def emit_norm(w, xe, xo):
                den = misc.tile([128, 8, 1], f32, tag="den")
                nc.vector.tensor_copy(den[:, 0:4], xe[:, :, DH:VW])
                nc.vector.tensor_copy(den[:, 4:8], xo[:, :, DH:VW])
                rcp = misc.tile([128, 8, 1], f32, tag="rcp")
                nc.vector.reciprocal(rcp[:], den[:])
                xn = xn_pool.tile([128, 8, DH], bf16)
                for j in range(4):
                    nc.vector.tensor_scalar_mul(
                        xn[:, j, :], xe[:, j, 0:DH], rcp[:, j])
                    nc.vector.tensor_scalar_mul(
                        xn[:, 4 + j, :], xo[:, j, 0:DH], rcp[:, 4 + j])
                return xn

            def emit_transpose(w, xn):
                tp = ps1.tile([128, 512], bf16, tag="ps", name="tp")
                for j in range(4):
                    nc.tensor.transpose(
                        tp[0:64, 128 * j:128 * (j + 1)], xn[:, j, :],
                        ident[:])
                    nc.tensor.transpose(
                        tp[64:128, 128 * j:128 * (j + 1)], xn[:, 4 + j, :],
                        ident[:])
                nc.vector.tensor_copy(xt_sb[:, w, :], tp[:])

            for w in range(8):
                # K^T chunk w (replicated: all 2048 key tokens)
                kt_w = kt_pool.tile([128, NT], bf16)
                for half in range(2):
                    acc = big_ps.tile([128, 1024], f32, tag="mm",
                                      name="kacc")
                    for h in range(2):
                        nsl = slice(512 * (2 * half + h),
                                    512 * (2 * half + h + 1))
                        for k in range(8):
                            nc.tensor.matmul(
                                acc[:, 512 * h:512 * (h + 1)],
                                wk_sb[:, k, 128 * w:128 * (w + 1)],
                                xk_sb[:, k, nsl],
                                start=(k == 0), stop=(k == 7))
                    nc.vector.tensor_copy(
                        kt_w[:, 1024 * half:1024 * (half + 1)],
                        acc[:])
                # Q^T chunk w (own 512 query tokens)
                acc = big_ps.tile([128, 1024], f32, tag="mm", name="qacc")
                for k in range(8):
                    nc.tensor.matmul(
                        acc[:, 0:512],
                        wq_sb[:, k, 128 * w:128 * (w + 1)],
                        xq_sb[:, k, :],
                        start=(k == 0), stop=(k == 7))
                nc.vector.tensor_copy(qt_sb[:, w, :], acc[:, 0:512])

                he, ho = 2 * w, 2 * w + 1
                xat_e = ps1.tile([128, 512], f32, tag="ps", name="xae")
                xat_o = ps1.tile([128, 512], f32, tag="ps", name="xao")
                xe = xat_e[:, 0:4 * VW].rearrange("p (j c) -> p j c", c=VW)
                xo = xat_o[:, 0:4 * VW].rearrange("p (j c) -> p j c", c=VW)
                for kt in range(NKT):
                    ksl = slice(128 * kt, 128 * (kt + 1))
                    st = big_ps.tile([128, 1024], f32, tag="mm", name="st")
                    nc.tensor.matmul(
                        st[:, 0:512],
                        kt_w[0:64, ksl], qt_sb[0:64, w, :],
                        tile_position=(0, 0))
                    nc.tensor.matmul(
                        st[:, 512:1024],
                        kt_w[64:128, ksl], qt_sb[64:128, w, :],
                        tile_position=(64, 0))
                    pt = pt_pool.tile([128, 1024], bf16)
                    nc.scalar.activation(pt[:], st[:], EXP, scale=SCALE)
                    # PSUM has_written is cleared BANK-wide by start=True:
                    # only the first matmul per xa bank carries start.
                    for j in range(4):
                        nc.tensor.matmul(
                            xe[:, j, :],
                            pt[:, 128 * j:128 * (j + 1)],
                            v_sb[:, kt, he, :],
                            start=(kt == 0 and j == 0),
                            stop=(kt == NKT - 1 and j == 3),
                            skip_group_check=True)
                        nc.tensor.matmul(
                            xo[:, j, :],
                            pt[:, 512 + 128 * j:512 + 128 * (j + 1)],
                            v_sb[:, kt, ho, :],
                            start=(kt == 0 and j == 0),
                            stop=(kt == NKT - 1 and j == 3),
                            skip_group_check=True)
                    if kt == 3 and pending is not None:
                        emit_transpose(*pending)
                        pending = None
                xn = emit_norm(w, xe, xo)
                pending = (w, xn)

            # wo/bias: queued behind the last K-proj reads (wpool slot A)
            wo_sb = wpool.tile([128, 8, D], bf16, tag="w", name="wo_sb")
            nc.sync.dma_start(out=wo_sb[:], in_=wo3[:])
            bias_sb = const.tile([128, D], f32)
            nc.sync.dma_start(out=bias_sb[:],
                              in_=bo[:].partition_broadcast(128))

            # --- output projection + bias ---------------------------------
            # Phase A (x-chunks 0..6 + bias -> psb) runs on PE right after
            # wave 7's PV while wave 7 normalizes on DVE; then wave 7's
            # transposes; then phase B (chunk 7) finishes each tile.
            for m in range(4):
                tsl = slice(128 * m, 128 * (m + 1))
                acc = big_ps.tile([128, 1024], f32, tag="mm", name="opp")
                for n in range(2):
                    osl = slice(512 * n, 512 * (n + 1))
                    for k in range(7):
                        nc.tensor.matmul(
                            acc[:, 512 * n:512 * (n + 1)],
                            xt_sb[:, k, tsl],
                            wo_sb[:, k, osl],
                            start=(k == 0), stop=(k == 6))
                for n in range(2):
                    nc.vector.tensor_add(
                        psb[:, 2 * m + n, :],
                        acc[:, 512 * n:512 * (n + 1)],
                        bias_sb[:, 512 * n:512 * (n + 1)])

            assert pending is not None
            emit_transpose(*pending)
            pending = None

            for m in range(4):
                tsl = slice(128 * m, 128 * (m + 1))
                for n in range(2):
                    osl = slice(512 * n, 512 * (n + 1))
                    acc = ps1.tile([128, 512], f32, tag="ps", name="opf")
                    nc.tensor.matmul(
                        acc[:], xt_sb[:, 7, tsl], wo_sb[:, 7, osl])
                    ob = outsb.tile([128, 512], f32)
                    nc.vector.tensor_add(ob[:], acc[:],
                                         psb[:, 2 * m + n, :])
                    nc.sync.dma_start(out=out[tsl, osl], in_=ob[:])

    return nc


_CACHE = {}


def _get_program():
    if "nc" not in _CACHE:
        _CACHE["nc"] = build_program()
    return _CACHE["nc"]


def _pre3(mat_t):
    """[D, X] (row = (t p)) -> contiguous [128, 8, X]."""
    x = mat_t.shape[1]
    return np.ascontiguousarray(
        mat_t.reshape(8, 128, x).transpose(1, 0, 2))


def make_in_maps(query, key, value, Wq, Wk, Wv, Wo, bo):
    """Host-side sharding: per-core input dicts (bf16)."""
    import ml_dtypes
    bf = ml_dtypes.bfloat16

    def b(x):
        return np.asarray(x, dtype=np.float32).astype(bf)

    query = np.asarray(query, dtype=np.float32)
    key = np.asarray(key, dtype=np.float32)
    value = np.asarray(value, dtype=np.float32)
    wq3 = _pre3(b(np.asarray(Wq, dtype=np.float32).T))
    wk3 = _pre3(b(np.asarray(Wk, dtype=np.float32).T))
    wv3 = _pre3(b(np.asarray(Wv, dtype=np.float32).T))
    wo3 = _pre3(b(np.asarray(Wo, dtype=np.float32).T))
    bo32 = np.ascontiguousarray(np.asarray(bo, dtype=np.float32))
    xk3 = [_pre3(b(key[bb].T)) for bb in range(B)]

    in_maps = []
    for c in range(NCORES):
        bb, g = divmod(c, CPB)
        qsl = slice(QSL * g, QSL * (g + 1))
        in_maps.append({
            "xq3": _pre3(b(query[bb, qsl, :].T)),
            "xk3": xk3[bb],
            "xv3": _pre3(b(value[bb, qsl, :].T)),
            "wq3": wq3,
            "wk3": wk3,
            "wv3": wv3,
            "wo3": wo3,
            "bo": bo32,
            "bsel": np.array([float(bb)], dtype=np.float32),
        })
    return in_maps


def assemble(results):
    """Concatenate per-core token slices into [B, NT, D]."""
    out = np.empty((B, NT, D), dtype=np.float32)
    for c in range(NCORES):
        bb, g = divmod(c, CPB)
        out[bb, QSL * g:QSL * (g + 1), :] = results[c]["out"]
    return out


def run(query, key, value, Wq, Wk, Wv, Wo, bo, trace=False):
    from concourse.bass_utils import run_bass_kernel_spmd
    nc = _get_program()
    in_maps = make_in_maps(query, key, value, Wq, Wk, Wv, Wo, bo)
    res = run_bass_kernel_spmd(nc, in_maps, core_ids=list(range(NCORES)),
                               trace=trace)
    return assemble(res.results), res


def kernel(query, key, value, qpos=None, kpos=None, Wq=None, Wk=None,
           Wv=None, Wo=None, bo=None):
    out, _ = run(query, key, value, Wq, Wk, Wv, Wo, bo)
    return out
